# revision 20
# baseline (speedup 1.0000x reference)
"""Trainium2 Bass kernel for a BasicTransformerBlock (B=2, S=2048, H=768, FF=3072, NH=12).

Sharding: core c handles batch b=c//4, sequence quarter q=c%4 (512 tokens).
Each core redundantly computes LN1 + K/V projections for its batch's full
2048 tokens (no collectives needed); Q/attention/Wo/FFN only for its own 512
tokens.  Activations are kept feature-major ([feature, token]) on chip;
attention scores are computed transposed ([tk, tq]) so softmax reduces over
the partition dim; the ones-column appended to V makes each ctx matmul also
accumulate the softmax denominator (row 64) for free.

LN affine params and all biases are folded host-side:
  Wq_eff = diag(ln1_w) Wq, bq_eff = ln1_b@Wq + bq  (same k)
  v carries no bias on device;  bo_eff = (ln1_b@Wv + bv)@Wo + bo
  W1_eff = diag(ln2_w) W1, b1_eff = ln2_b@W1 + b1
"""

import numpy as np
import ml_dtypes

import concourse.bass as bass
import concourse.tile as tile
from concourse import bacc, mybir
from concourse.bass import ts, ds
from concourse.alu_op_type import AluOpType
from concourse.bass_utils import run_bass_kernel_spmd

F32 = mybir.dt.float32
BF16 = mybir.dt.bfloat16
AF = mybir.ActivationFunctionType

H = 768
FF = 3072
NH = 12
DH = 64
B = 2
S = 2048
P = 128
NCORES = 8
TQ = 512          # own tokens per core
NTT = S // TQ     # 4 token tiles per batch
FC = H // P       # 6 feature chunks
FFC = FF // P     # 24 hidden chunks
TKC = S // P      # 16 key token chunks
HPAIRS = NH // 2  # 6 head pairs
EPS = 1e-6


def _dma_bcast(nc, out_tile, row_ap, nparts, T):
    """Replicate a [1,T] SBUF row across nparts partitions via DMA."""
    nc.sync.dma_start(out_tile[:],
                      row_ap.unsqueeze(1).broadcast_to((1, nparts, T)))


def _ln_tail(nc, T, ps_sum, ps_sq, small_pool, ab_pool, eps_tile):
    """From accumulated sum (partition 0) / sqsum (partition 32) rows ->
    broadcast alpha/beta [P,T] tiles."""
    mu = small_pool.tile([1, T], F32, tag="lnsmall")
    nc.vector.tensor_scalar_mul(mu[:], ps_sum, 1.0 / H)
    msq = small_pool.tile([1, T], F32, tag="lnsmall")
    nc.vector.tensor_scalar_mul(msq[:], ps_sq, 1.0 / H)
    var = small_pool.tile([1, T], F32, tag="lnsmall")
    nc.vector.tensor_mul(var[:], mu[:], mu[:])
    nc.vector.tensor_sub(var[:], msq[:], var[:])
    sd = small_pool.tile([1, T], F32, tag="lnsmall")
    nc.scalar.activation(sd[:], var[:], AF.Sqrt, bias=eps_tile[:])
    rsig = small_pool.tile([1, T], F32, tag="lnsmall")
    nc.vector.reciprocal(rsig[:], sd[:])
    beta = small_pool.tile([1, T], F32, tag="lnsmall")
    nc.vector.scalar_tensor_tensor(beta[:], mu[:], -1.0, rsig[:],
                                   AluOpType.mult, AluOpType.mult)
    ab = ab_pool.tile([P, T], F32, tag="ab")
    _dma_bcast(nc, ab, rsig[0:1, :], P, T)
    bb = ab_pool.tile([P, T], F32, tag="bb")
    _dma_bcast(nc, bb, beta[0:1, :], P, T)
    return ab, bb


def build():
    nc = bacc.Bacc("TRN2", target_bir_lowering=False, debug=False,
                   num_devices=NCORES)

    latq_d = nc.dram_tensor("latTq", [H, TQ], F32, kind="ExternalInput")
    latbf_d = nc.dram_tensor("latTbf", [H, S], BF16, kind="ExternalInput")
    wq_d = nc.dram_tensor("wq", [H, H], BF16, kind="ExternalInput")
    wk_d = nc.dram_tensor("wk", [H, H], BF16, kind="ExternalInput")
    wv_d = nc.dram_tensor("wv", [H, H], BF16, kind="ExternalInput")
    wo_d = nc.dram_tensor("wo", [H, H], BF16, kind="ExternalInput")
    w1_d = nc.dram_tensor("w1r", [FFC, H, P], BF16, kind="ExternalInput")
    w2_d = nc.dram_tensor("w2", [FF, H], BF16, kind="ExternalInput")
    bq_d = nc.dram_tensor("bq", [P, FC], F32, kind="ExternalInput")
    bk_d = nc.dram_tensor("bk", [P, FC], F32, kind="ExternalInput")
    bo_d = nc.dram_tensor("bo", [P, FC], F32, kind="ExternalInput")
    b1_d = nc.dram_tensor("b1", [P, FFC], F32, kind="ExternalInput")
    b2_d = nc.dram_tensor("b2", [P, FC], F32, kind="ExternalInput")
    out_d = nc.dram_tensor("outT", [H, TQ], F32, kind="ExternalOutput")

    latq_ap = latq_d.ap().rearrange("(c p) t -> p c t", p=P)
    latbf_ap = latbf_d.ap().rearrange("(c p) t -> p c t", p=P)
    out_ap = out_d.ap().rearrange("(c p) t -> p c t", p=P)

    with tile.TileContext(nc) as tc:
        with (
            tc.tile_pool(name="consts", bufs=1) as consts,
            tc.tile_pool(name="persist", bufs=1) as persist,
        ):
            # constants
            ones_col_bf = consts.tile([P, 1], BF16)
            nc.vector.memset(ones_col_bf[:], 1.0)
            eps_tile = consts.tile([1, 1], F32)
            nc.vector.memset(eps_tile[:], EPS)
            zero_col = consts.tile([P, 1], F32)
            nc.vector.memset(zero_col[:], 0.0)
            bq_sb = consts.tile([P, FC], F32)
            nc.sync.dma_start(bq_sb[:], bq_d.ap())
            bk_sb = consts.tile([P, FC], F32)
            nc.sync.dma_start(bk_sb[:], bk_d.ap())
            bo_sb = consts.tile([P, FC], F32)
            nc.sync.dma_start(bo_sb[:], bo_d.ap())
            b1_sb = consts.tile([P, FFC], F32)
            nc.sync.dma_start(b1_sb[:], b1_d.ap())
            b2_sb = consts.tile([P, FC], F32)
            nc.sync.dma_start(b2_sb[:], b2_d.ap())

            # persistent activations (split into per-slice tiles so consumers
            # depend only on the pieces they read)
            kT = []
            for t in range(NTT):
                kT_t = persist.tile([P, FC, TQ], BF16, tag=f"kT{t}")
                kT.append(kT_t)
            v_sb = persist.tile([P, TKC, NH, DH + 1], BF16)
            nc.vector.memset(v_sb[:, :, :, DH:DH + 1], 1.0)
            qT = persist.tile([P, FC, TQ], BF16)
            ctxT = []
            for hh in range(HPAIRS):
                ctxT_h = persist.tile([P, TQ], BF16, tag=f"ctxT{hh}")
                ctxT.append(ctxT_h)
            resid1 = persist.tile([P, FC, TQ], F32)

            # projection weights (scalar-ring DMA so latT loads aren't queued
            # behind them on the sync HWDGE FIFO)
            wq_sb = persist.tile([P, FC, H], BF16)
            nc.scalar.dma_start(wq_sb[:], wq_d.ap().rearrange("(c p) m -> p c m", p=P))
            wk_sb = persist.tile([P, FC, H], BF16)
            nc.scalar.dma_start(wk_sb[:], wk_d.ap().rearrange("(c p) m -> p c m", p=P))
            wv_sb = persist.tile([P, FC, H], BF16)
            nc.scalar.dma_start(wv_sb[:], wv_d.ap().rearrange("(c p) m -> p c m", p=P))
            wo_sb = persist.tile([P, FC, H], BF16)
            nc.scalar.dma_start(wo_sb[:], wo_d.ap().rearrange("(c p) m -> p c m", p=P))

            # ---------------- Phases 1+2 under shared attention pools ------
            attn_outer = tc.tile_pool(name="attnp", bufs=4)
            attnp = attn_outer.__enter__()
            sc_outer = tc.tile_pool(name="ps_sc", bufs=2, space="PSUM")
            ps_sc = sc_outer.__enter__()
            # ---------------- Phase 1: LN1 + K/V/Q projections ----------------
            with (
                tc.tile_pool(name="latp", bufs=2) as latp,
                tc.tile_pool(name="sqp", bufs=2) as sqp,
                tc.tile_pool(name="nxp", bufs=2) as nxp,
                tc.tile_pool(name="abp", bufs=2) as abp,
                tc.tile_pool(name="smallp", bufs=8) as smallp,
                tc.tile_pool(name="lntmpp", bufs=2) as lntmpp,
                tc.tile_pool(name="ps_stats", bufs=2, space="PSUM") as ps_stats,
                tc.tile_pool(name="ps_kq", bufs=1, space="PSUM") as ps_kq,
                tc.tile_pool(name="ps_v", bufs=1, space="PSUM") as ps_v,
            ):
                nc.sync.dma_start(resid1[:], latq_ap)
                for tt in range(NTT):
                    latbf_t = latp.tile([P, FC, TQ], BF16, tag="latbf")
                    nc.sync.dma_start(latbf_t[:], latbf_ap[:, :, ts(tt, TQ)])

                    # LN1 stats via M=1 ones-matmul reductions
                    sq_t = sqp.tile([P, FC, TQ], BF16, tag="sq")
                    ps_stat = ps_stats.tile([33, TQ], F32, tag="stats")
                    for c in range(FC):
                        nc.tensor.matmul(ps_stat[0:1, :], ones_col_bf[:],
                                         latbf_t[:, c, :],
                                         start=(c == 0), stop=(c == FC - 1))
                    nc.vector.tensor_mul(sq_t[:], latbf_t[:], latbf_t[:])
                    for c in range(FC):
                        nc.tensor.matmul(ps_stat[32:33, :], ones_col_bf[:],
                                         sq_t[:, c, :],
                                         start=(c == 0), stop=(c == FC - 1))
                    ab, bb = _ln_tail(nc, TQ, ps_stat[0:1, :], ps_stat[32:33, :],
                                      smallp, abp, eps_tile)
                    nx_t = nxp.tile([P, FC, TQ], BF16, tag="nx")
                    for c in range(FC):
                        t = lntmpp.tile([P, TQ], F32, tag="lntmp")
                        nc.vector.tensor_mul(t[:], latbf_t[:, c, :], ab[:])
                        nc.vector.tensor_add(nx_t[:, c, :], t[:], bb[:])

                    # K projection (feature-major out)
                    for mc in range(FC):
                        ps = ps_kq.tile([P, TQ], F32, tag="kq")
                        for kc in range(FC):
                            nc.tensor.matmul(ps[:], wk_sb[:, kc, ts(mc, P)],
                                             nx_t[:, kc, :],
                                             start=(kc == 0), stop=(kc == FC - 1))
                        nc.scalar.activation(kT[tt][:, mc, :], ps[:],
                                             AF.Identity, bias=bk_sb[:, mc:mc + 1])
                    # V projection (token-major out, ones col preset)
                    for tcl in range(TQ // P):
                        tcg = tt * (TQ // P) + tcl
                        for half in range(2):
                            ps = ps_v.tile([P, 384], F32, tag="v")
                            for kc in range(FC):
                                nc.tensor.matmul(ps[:], nx_t[:, kc, ts(tcl, P)],
                                                 wv_sb[:, kc, ds(half * 384, 384)],
                                                 start=(kc == 0), stop=(kc == FC - 1))
                            nc.vector.tensor_copy(
                                v_sb[:, tcg, ds(half * 6, 6), 0:DH],
                                ps[:].rearrange("p (h d) -> p h d", d=DH))
                    # Q projection (own tokens live in tt==0)
                    if tt == 0:
                        for mc in range(FC):
                            ps = ps_kq.tile([P, TQ], F32, tag="kq")
                            for kc in range(FC):
                                nc.tensor.matmul(ps[:], wq_sb[:, kc, ts(mc, P)],
                                                 nx_t[:, kc, :],
                                                 start=(kc == 0), stop=(kc == FC - 1))
                            nc.scalar.activation(qT[:, mc, :], ps[:],
                                                 AF.Identity, bias=bq_sb[:, mc:mc + 1])

            # ---------------- Phase 2: attention ----------------
            # Two head-pairs interleaved; per chunk j the pair's scores land in
            # one [P,2,TQ] PSUM tile (heads row-tiled, concurrent), one Exp
            # evicts both; ctx matmuls are M=65 (ones column) so row 64
            # accumulates the softmax denominator.  Tails evict unnormalized
            # ctx to SBUF fast and normalize asynchronously.
            with (
                tc.tile_pool(name="rssb", bufs=4) as rssb,
                tc.tile_pool(name="rbp", bufs=4) as rbp,
                tc.tile_pool(name="stgp", bufs=4) as stgp,
                tc.tile_pool(name="ps_ctx", bufs=1, space="PSUM") as ps_ctx,
            ):
                for hpg in range(HPAIRS // 2):
                    hps = (2 * hpg, 2 * hpg + 1)
                    ctx_tiles = {}
                    for hp in hps:
                        ctxA_ps = ps_ctx.tile([DH + 1, TQ], F32, tag=f"ctxA{hp % 2}")
                        ctxB_ps = ps_ctx.tile([DH + 1, TQ], F32, tag=f"ctxB{hp % 2}")
                        ctx_tiles[hp] = (ctxA_ps, ctxB_ps)
                    for j in range(TKC):
                        jt, jj = j // (TQ // P), j % (TQ // P)
                        for hp in hps:
                            hA, hB = 2 * hp, 2 * hp + 1
                            sc = ps_sc.tile([P, 2, TQ], F32, tag="sc")
                            nc.tensor.matmul(sc[:, 0, :],
                                             kT[jt][0:DH, hp, ts(jj, P)],
                                             qT[0:DH, hp, :],
                                             start=True, stop=True)
                            nc.tensor.matmul(sc[:, 1, :],
                                             kT[jt][DH:P, hp, ts(jj, P)],
                                             qT[DH:P, hp, :],
                                             start=True, stop=True)
                            a2 = attnp.tile([P, 2, TQ], BF16, tag="attn")
                            nc.scalar.activation(a2[:], sc[:], AF.Exp, scale=0.125,
                                                 bias=zero_col[:])
                            ctxA_ps, ctxB_ps = ctx_tiles[hp]
                            nc.tensor.matmul(ctxA_ps[:], v_sb[:, j, hA, :],
                                             a2[:, 0, :],
                                             start=(j == 0), stop=(j == TKC - 1))
                            nc.tensor.matmul(ctxB_ps[:], v_sb[:, j, hB, :],
                                             a2[:, 1, :],
                                             start=(j == 0), stop=(j == TKC - 1))
                    for hp in hps:
                        ctxA_ps, ctxB_ps = ctx_tiles[hp]
                        cuA = stgp.tile([DH + 1, TQ], F32, tag="cuA")
                        nc.scalar.copy(cuA[:], ctxA_ps[:])
                        cuB = stgp.tile([DH + 1, TQ], F32, tag="cuB")
                        nc.scalar.copy(cuB[:], ctxB_ps[:])
                        rcA = rssb.tile([DH + 1, TQ], F32, tag="rcA")
                        nc.vector.reciprocal(rcA[DH:DH + 1, :], cuA[DH:DH + 1, :])
                        rcB = rssb.tile([DH + 1, TQ], F32, tag="rcB")
                        nc.vector.reciprocal(rcB[DH:DH + 1, :], cuB[DH:DH + 1, :])
                        rbA = rbp.tile([DH, TQ], F32, tag="rbA")
                        _dma_bcast(nc, rbA, rcA[DH:DH + 1, :], DH, TQ)
                        rbB = rbp.tile([DH, TQ], F32, tag="rbB")
                        _dma_bcast(nc, rbB, rcB[DH:DH + 1, :], DH, TQ)
                        nc.vector.tensor_mul(ctxT[hp][0:DH, :], cuA[0:DH, :],
                                             rbA[:])
                        stgB = stgp.tile([DH, TQ], BF16, tag="stgB")
                        nc.vector.tensor_mul(stgB[:], cuB[0:DH, :], rbB[:])
                        nc.sync.dma_start(ctxT[hp][DH:P, :], stgB[:])

            sc_outer.__exit__(None, None, None)
            attn_outer.__exit__(None, None, None)

            # ---------------- Phase 3: Wo + LN2 + FFN ----------------
            with (
                tc.tile_pool(name="lat2p", bufs=1) as lat2p,
                tc.tile_pool(name="nx2p", bufs=1) as nx2p,
                tc.tile_pool(name="sq2p", bufs=1) as sq2p,
                tc.tile_pool(name="ab2p", bufs=1) as ab2p,
                tc.tile_pool(name="small2p", bufs=8) as small2p,
                tc.tile_pool(name="lntmp2p", bufs=2) as lntmp2p,
                tc.tile_pool(name="w1sp", bufs=4) as w1sp,
                tc.tile_pool(name="w2sp", bufs=4) as w2sp,
                tc.tile_pool(name="hp_pool", bufs=4) as hp_pool,
                tc.tile_pool(name="outp", bufs=1) as outp,
            ):
                lat2T = lat2p.tile([P, FC, TQ], F32)
                nx2T = []
                for cc in range(FC):
                    nx2T_c = nx2p.tile([P, TQ], BF16, tag=f"nx2T{cc}")
                    nx2T.append(nx2T_c)
                with (
                    tc.tile_pool(name="ps_wo", bufs=2, space="PSUM") as ps_wo,
                    tc.tile_pool(name="ps_st2", bufs=1, space="PSUM") as ps_st2,
                ):
                    # Wo projection + residual, LN2 stats interleaved per-chunk
                    sq2 = sq2p.tile([P, FC, TQ], BF16, tag="sq2")
                    latbf2 = sq2p.tile([P, FC, TQ], BF16, tag="latbf2")
                    ps_sum2 = ps_st2.tile([1, TQ], F32, tag="sum2")
                    ps_sq2 = ps_st2.tile([33, TQ], F32, tag="sqs2")
                    for mc in range(FC):
                        ps = ps_wo.tile([P, TQ], F32, tag="wo")
                        for kc in range(FC):
                            nc.tensor.matmul(ps[:], wo_sb[:, kc, ts(mc, P)],
                                             ctxT[kc][:],
                                             start=(kc == 0), stop=(kc == FC - 1))
                        nc.vector.affine_then_add(lat2T[:, mc, :], ps[:],
                                                  resid1[:, mc, :], 1.0,
                                                  bo_sb[:, mc:mc + 1])
                        nc.scalar.copy(latbf2[:, mc, :], lat2T[:, mc, :])
                        nc.vector.tensor_mul(sq2[:, mc, :], lat2T[:, mc, :],
                                             lat2T[:, mc, :])
                        nc.tensor.matmul(ps_sum2[0:1, :], ones_col_bf[:],
                                         latbf2[:, mc, :],
                                         start=(mc == 0), stop=(mc == FC - 1))
                        nc.tensor.matmul(ps_sq2[32:33, :], ones_col_bf[:],
                                         sq2[:, mc, :],
                                         start=(mc == 0), stop=(mc == FC - 1))
                    ab2, bb2 = _ln_tail(nc, TQ, ps_sum2[0:1, :],
                                        ps_sq2[32:33, :], small2p, ab2p,
                                        eps_tile)
                    for c in range(FC):
                        t2 = lntmp2p.tile([P, TQ], F32, tag="lntmp2")
                        nc.vector.tensor_mul(t2[:], lat2T[:, c, :], ab2[:])
                        nc.vector.tensor_add(nx2T[c][:], t2[:], bb2[:])

                outT = outp.tile([P, FC, TQ], F32)
                with (
                    tc.tile_pool(name="ps_fo", bufs=1, space="PSUM") as ps_fo,
                    tc.tile_pool(name="ps_h", bufs=2, space="PSUM") as ps_h,
                ):
                    ps_out = ps_fo.tile([P, FC, TQ], F32)
                    for mh in range(FFC):
                        w1t = w1sp.tile([P, FC, P], BF16, tag="w1s")
                        nc.sync.dma_start(
                            w1t[:], w1_d.ap()[mh].rearrange("(c p) m -> p c m", p=P))
                        w2t = w2sp.tile([P, H], BF16, tag="w2s")
                        nc.sync.dma_start(w2t[:], w2_d.ap()[ts(mh, P)])
                        psh = ps_h.tile([P, TQ], F32, tag="h")
                        for kc in range(FC):
                            nc.tensor.matmul(psh[:], w1t[:, kc, :], nx2T[kc][:],
                                             start=(kc == 0), stop=(kc == FC - 1))
                        h_t = hp_pool.tile([P, TQ], BF16, tag="h_sb")
                        nc.scalar.activation(h_t[:], psh[:], AF.Gelu,
                                             bias=b1_sb[:, mh:mh + 1])
                        for mc in range(FC):
                            nc.tensor.matmul(ps_out[:, mc, :], w2t[:, ts(mc, P)],
                                             h_t[:],
                                             start=(mh == 0), stop=(mh == FFC - 1))
                    for mc in range(FC):
                        nc.vector.affine_then_add(outT[:, mc, :], ps_out[:, mc, :],
                                                  lat2T[:, mc, :], 1.0,
                                                  b2_sb[:, mc:mc + 1])
                nc.sync.dma_start(out_ap, outT[:])

    nc.compile()
    return nc


_NC_CACHE = {}


def _get_nc():
    if "nc" not in _NC_CACHE:
        _NC_CACHE["nc"] = build()
    return _NC_CACHE["nc"]


def _prep_inputs(latent, ln1_w, ln1_b, Wq, bq, Wk, bk, Wv, bv, Wo, bo,
                 ln2_w, ln2_b, W1, b1, W2, b2):
    f32 = np.float32
    bf16 = ml_dtypes.bfloat16
    lat = np.asarray(latent, f32)
    ln1_w = np.asarray(ln1_w, f32); ln1_b = np.asarray(ln1_b, f32)
    ln2_w = np.asarray(ln2_w, f32); ln2_b = np.asarray(ln2_b, f32)
    Wq = np.asarray(Wq, f32); Wk = np.asarray(Wk, f32); Wv = np.asarray(Wv, f32)
    Wo = np.asarray(Wo, f32); W1 = np.asarray(W1, f32); W2 = np.asarray(W2, f32)
    bq = np.asarray(bq, f32); bk = np.asarray(bk, f32); bv = np.asarray(bv, f32)
    bo = np.asarray(bo, f32); b1 = np.asarray(b1, f32); b2 = np.asarray(b2, f32)

    wq_eff = (ln1_w[:, None] * Wq).astype(bf16)
    wk_eff = (ln1_w[:, None] * Wk).astype(bf16)
    wv_eff = (ln1_w[:, None] * Wv).astype(bf16)
    wo_bf = Wo.astype(bf16)
    bq_eff = ln1_b @ Wq + bq
    bk_eff = ln1_b @ Wk + bk
    bv_eff = ln1_b @ Wv + bv
    bo_eff = bv_eff @ Wo + bo
    w1_eff = ln2_w[:, None] * W1
    b1_eff = ln2_b @ W1 + b1
    w1r = np.ascontiguousarray(
        w1_eff.reshape(H, FFC, P).transpose(1, 0, 2)).astype(bf16)
    w2_bf = W2.astype(bf16)

    def chunked(b):  # [H or FF] -> [P, nchunks]
        return np.ascontiguousarray(b.reshape(-1, P).T)

    common = {
        "wq": wq_eff, "wk": wk_eff, "wv": wv_eff, "wo": wo_bf,
        "w1r": w1r, "w2": w2_bf,
        "bq": chunked(bq_eff), "bk": chunked(bk_eff), "bo": chunked(bo_eff),
        "b1": chunked(b1_eff), "b2": chunked(b2),
    }
    in_maps = []
    for c in range(NCORES):
        b = c // (NCORES // B)
        q = c % (NCORES // B)
        latT_c = np.ascontiguousarray(np.roll(lat[b].T, -q * TQ, axis=1))
        m = dict(common)
        m["latTq"] = np.ascontiguousarray(latT_c[:, :TQ])
        m["latTbf"] = latT_c.astype(bf16)
        in_maps.append(m)
    return in_maps


def kernel(**inputs):
    nc = _get_nc()
    in_maps = _prep_inputs(**inputs)
    res = run_bass_kernel_spmd(nc, in_maps, core_ids=list(range(NCORES)))
    out = np.empty((B, S, H), np.float32)
    for c in range(NCORES):
        b = c // (NCORES // B)
        q = c % (NCORES // B)
        out[b, q * TQ:(q + 1) * TQ, :] = res.results[c]["outT"].T
    return out


# revision 21
# speedup vs baseline: 1.2289x; 1.2289x over previous
"""Trainium2 Bass kernel for a BasicTransformerBlock (B=2, S=2048, H=768, FF=3072, NH=12).

Sharding: core c handles batch b=c//4, sequence quarter q=c%4 (512 tokens).
Each core redundantly computes LN1 + K/V projections for its batch's full
2048 tokens (no collectives needed); Q/attention/Wo/FFN only for its own 512
tokens.  Activations are kept feature-major ([feature, token]) on chip;
attention scores are computed transposed ([tk, tq]) so softmax reduces over
the partition dim; the ones-column appended to V makes each ctx matmul also
accumulate the softmax denominator (row 64) for free.

LN affine params and all biases are folded host-side:
  Wq_eff = diag(ln1_w) Wq, bq_eff = ln1_b@Wq + bq  (same k)
  v carries no bias on device;  bo_eff = (ln1_b@Wv + bv)@Wo + bo
  W1_eff = diag(ln2_w) W1, b1_eff = ln2_b@W1 + b1
"""

import numpy as np
import ml_dtypes

import concourse.bass as bass
import concourse.tile as tile
from concourse import bacc, mybir
from concourse.bass import ts, ds
from concourse.alu_op_type import AluOpType
from concourse.bass_utils import run_bass_kernel_spmd

F32 = mybir.dt.float32
BF16 = mybir.dt.bfloat16
AF = mybir.ActivationFunctionType

H = 768
FF = 3072
NH = 12
DH = 64
B = 2
S = 2048
P = 128
NCORES = 8
TQ = 512          # own tokens per core
NTT = S // TQ     # 4 token tiles per batch
FC = H // P       # 6 feature chunks
FFC = FF // P     # 24 hidden chunks
TKC = S // P      # 16 key token chunks
HPAIRS = NH // 2  # 6 head pairs
EPS = 1e-6


def _dma_bcast(nc, out_tile, row_ap, nparts, T):
    """Replicate a [1,T] SBUF row across nparts partitions via DMA."""
    nc.sync.dma_start(out_tile[:],
                      row_ap.unsqueeze(1).broadcast_to((1, nparts, T)))


def _ln_tail(nc, T, ps_sum, ps_sq, small_pool, ab_pool, eps_tile):
    """From accumulated sum (partition 0) / sqsum (partition 32) rows ->
    broadcast alpha/beta [P,T] tiles."""
    mu = small_pool.tile([1, T], F32, tag="lnsmall")
    nc.vector.tensor_scalar_mul(mu[:], ps_sum, 1.0 / H)
    msq = small_pool.tile([1, T], F32, tag="lnsmall")
    nc.vector.tensor_scalar_mul(msq[:], ps_sq, 1.0 / H)
    var = small_pool.tile([1, T], F32, tag="lnsmall")
    nc.vector.tensor_mul(var[:], mu[:], mu[:])
    nc.vector.tensor_sub(var[:], msq[:], var[:])
    sd = small_pool.tile([1, T], F32, tag="lnsmall")
    nc.scalar.activation(sd[:], var[:], AF.Sqrt, bias=eps_tile[:])
    rsig = small_pool.tile([1, T], F32, tag="lnsmall")
    nc.vector.reciprocal(rsig[:], sd[:])
    beta = small_pool.tile([1, T], F32, tag="lnsmall")
    nc.vector.scalar_tensor_tensor(beta[:], mu[:], -1.0, rsig[:],
                                   AluOpType.mult, AluOpType.mult)
    ab = ab_pool.tile([P, T], F32, tag="ab")
    nc.gpsimd.partition_broadcast(ab[:], rsig[0:1, :])
    bb = ab_pool.tile([P, T], F32, tag="bb")
    nc.gpsimd.partition_broadcast(bb[:], beta[0:1, :])
    return ab, bb


def build():
    nc = bacc.Bacc("TRN2", target_bir_lowering=False, debug=False,
                   num_devices=NCORES)

    latq_d = nc.dram_tensor("latTq", [H, TQ], F32, kind="ExternalInput")
    latbf_d = nc.dram_tensor("latTbf", [H, S], BF16, kind="ExternalInput")
    wq_d = nc.dram_tensor("wq", [H, H], BF16, kind="ExternalInput")
    wk_d = nc.dram_tensor("wk", [H, H], BF16, kind="ExternalInput")
    wv_d = nc.dram_tensor("wv", [H, H], BF16, kind="ExternalInput")
    wo_d = nc.dram_tensor("wo", [H, H], BF16, kind="ExternalInput")
    w1_d = nc.dram_tensor("w1r", [FFC, H, P], BF16, kind="ExternalInput")
    w2_d = nc.dram_tensor("w2", [FF, H], BF16, kind="ExternalInput")
    bq_d = nc.dram_tensor("bq", [P, FC], F32, kind="ExternalInput")
    bk_d = nc.dram_tensor("bk", [P, FC], F32, kind="ExternalInput")
    bo_d = nc.dram_tensor("bo", [P, FC], F32, kind="ExternalInput")
    b1_d = nc.dram_tensor("b1", [P, FFC], F32, kind="ExternalInput")
    b2_d = nc.dram_tensor("b2", [P, FC], F32, kind="ExternalInput")
    out_d = nc.dram_tensor("outT", [H, TQ], F32, kind="ExternalOutput")

    latq_ap = latq_d.ap().rearrange("(c p) t -> p c t", p=P)
    latbf_ap = latbf_d.ap().rearrange("(c p) t -> p c t", p=P)
    out_ap = out_d.ap().rearrange("(c p) t -> p c t", p=P)

    with tile.TileContext(nc) as tc:
        with (
            tc.tile_pool(name="consts", bufs=1) as consts,
            tc.tile_pool(name="persist", bufs=1) as persist,
        ):
            # constants
            ones_col_bf = consts.tile([P, 1], BF16)
            nc.vector.memset(ones_col_bf[:], 1.0)
            eps_tile = consts.tile([1, 1], F32)
            nc.vector.memset(eps_tile[:], EPS)
            zero_col = consts.tile([P, 1], F32)
            nc.vector.memset(zero_col[:], 0.0)
            bq_sb = consts.tile([P, FC], F32)
            nc.sync.dma_start(bq_sb[:], bq_d.ap())
            bk_sb = consts.tile([P, FC], F32)
            nc.sync.dma_start(bk_sb[:], bk_d.ap())
            bo_sb = consts.tile([P, FC], F32)
            nc.sync.dma_start(bo_sb[:], bo_d.ap())
            b1_sb = consts.tile([P, FFC], F32)
            nc.sync.dma_start(b1_sb[:], b1_d.ap())
            b2_sb = consts.tile([P, FC], F32)
            nc.sync.dma_start(b2_sb[:], b2_d.ap())

            # persistent activations (split into per-slice tiles so consumers
            # depend only on the pieces they read)
            kT = []
            for t in range(NTT):
                kT_t = persist.tile([P, FC, TQ], BF16, tag=f"kT{t}")
                kT.append(kT_t)
            v_sb = persist.tile([P, TKC, NH, DH + 1], BF16)
            nc.vector.memset(v_sb[:, :, :, DH:DH + 1], 1.0)
            qT = persist.tile([P, FC, TQ], BF16)
            ctxT = []
            for hh in range(HPAIRS):
                ctxT_h = persist.tile([P, TQ], BF16, tag=f"ctxT{hh}")
                ctxT.append(ctxT_h)
            resid1 = persist.tile([P, FC, TQ], F32)

            # projection weights (scalar-ring DMA so latT loads aren't queued
            # behind them on the sync HWDGE FIFO)
            wq_sb = persist.tile([P, FC, H], BF16)
            nc.scalar.dma_start(wq_sb[:], wq_d.ap().rearrange("(c p) m -> p c m", p=P))
            wk_sb = persist.tile([P, FC, H], BF16)
            nc.scalar.dma_start(wk_sb[:], wk_d.ap().rearrange("(c p) m -> p c m", p=P))
            wv_sb = persist.tile([P, FC, H], BF16)
            nc.scalar.dma_start(wv_sb[:], wv_d.ap().rearrange("(c p) m -> p c m", p=P))
            wo_sb = persist.tile([P, FC, H], BF16)
            nc.scalar.dma_start(wo_sb[:], wo_d.ap().rearrange("(c p) m -> p c m", p=P))

            # ---------------- Phase 1: LN1 + K/V/Q projections ----------------
            with (
                tc.tile_pool(name="latp", bufs=2) as latp,
                tc.tile_pool(name="sqp", bufs=2) as sqp,
                tc.tile_pool(name="nxp", bufs=2) as nxp,
                tc.tile_pool(name="abp", bufs=2) as abp,
                tc.tile_pool(name="smallp", bufs=12) as smallp,
                tc.tile_pool(name="lntmpp", bufs=2) as lntmpp,
                tc.tile_pool(name="ps_stats", bufs=2, space="PSUM") as ps_stats,
                tc.tile_pool(name="ps_kq", bufs=2, space="PSUM") as ps_kq,
                tc.tile_pool(name="ps_v", bufs=2, space="PSUM") as ps_v,
            ):
                nc.sync.dma_start(resid1[:], latq_ap)
                for tt in range(NTT):
                    latbf_t = latp.tile([P, FC, TQ], BF16, tag="latbf")
                    nc.sync.dma_start(latbf_t[:], latbf_ap[:, :, ts(tt, TQ)])

                    # LN1 stats via M=1 ones-matmul reductions
                    sq_t = sqp.tile([P, FC, TQ], BF16, tag="sq")
                    ps_stat = ps_stats.tile([33, TQ], F32, tag="stats")
                    for c in range(FC):
                        nc.tensor.matmul(ps_stat[0:1, :], ones_col_bf[:],
                                         latbf_t[:, c, :],
                                         start=(c == 0), stop=(c == FC - 1))
                    nc.vector.tensor_mul(sq_t[:], latbf_t[:], latbf_t[:])
                    for c in range(FC):
                        nc.tensor.matmul(ps_stat[32:33, :], ones_col_bf[:],
                                         sq_t[:, c, :],
                                         start=(c == 0), stop=(c == FC - 1))
                    ab, bb = _ln_tail(nc, TQ, ps_stat[0:1, :], ps_stat[32:33, :],
                                      smallp, abp, eps_tile)
                    nx_t = nxp.tile([P, FC, TQ], BF16, tag="nx")
                    for c in range(FC):
                        t = lntmpp.tile([P, TQ], F32, tag="lntmp")
                        nc.vector.tensor_mul(t[:], latbf_t[:, c, :], ab[:])
                        nc.vector.tensor_add(nx_t[:, c, :], t[:], bb[:])

                    # K projection (feature-major out)
                    for mc in range(FC):
                        ps = ps_kq.tile([P, TQ], F32, tag="kq")
                        for kc in range(FC):
                            nc.tensor.matmul(ps[:], wk_sb[:, kc, ts(mc, P)],
                                             nx_t[:, kc, :],
                                             start=(kc == 0), stop=(kc == FC - 1))
                        nc.scalar.activation(kT[tt][:, mc, :], ps[:],
                                             AF.Identity, bias=bk_sb[:, mc:mc + 1])
                    # V projection (token-major out, ones col preset)
                    for tcl in range(TQ // P):
                        tcg = tt * (TQ // P) + tcl
                        for half in range(2):
                            ps = ps_v.tile([P, 384], F32, tag="v")
                            for kc in range(FC):
                                nc.tensor.matmul(ps[:], nx_t[:, kc, ts(tcl, P)],
                                                 wv_sb[:, kc, ds(half * 384, 384)],
                                                 start=(kc == 0), stop=(kc == FC - 1))
                            nc.vector.tensor_copy(
                                v_sb[:, tcg, ds(half * 6, 6), 0:DH],
                                ps[:].rearrange("p (h d) -> p h d", d=DH))
                    # Q projection (own tokens live in tt==0)
                    if tt == 0:
                        for mc in range(FC):
                            ps = ps_kq.tile([P, TQ], F32, tag="kq")
                            for kc in range(FC):
                                nc.tensor.matmul(ps[:], wq_sb[:, kc, ts(mc, P)],
                                                 nx_t[:, kc, :],
                                                 start=(kc == 0), stop=(kc == FC - 1))
                            nc.scalar.activation(qT[:, mc, :], ps[:],
                                                 AF.Identity, bias=bq_sb[:, mc:mc + 1])

            # ---------------- Phase 2: attention ----------------
            # Two head-pairs interleaved; per chunk j the pair's scores land in
            # one [P,2,TQ] PSUM tile (heads row-tiled, concurrent), one Exp
            # evicts both; ctx matmuls are M=65 (ones column) so row 64
            # accumulates the softmax denominator.  Tails evict unnormalized
            # ctx to SBUF fast and normalize asynchronously.
            with (
                tc.tile_pool(name="attnp", bufs=6) as attnp,
                tc.tile_pool(name="rssb", bufs=2) as rssb,
                tc.tile_pool(name="rbp", bufs=4) as rbp,
                tc.tile_pool(name="stgp", bufs=2) as stgp,
                tc.tile_pool(name="ps_sc", bufs=2, space="PSUM") as ps_sc,
                tc.tile_pool(name="ps_ctx", bufs=1, space="PSUM") as ps_ctx,
            ):
                for hpg in range(HPAIRS // 2):
                    hps = (2 * hpg, 2 * hpg + 1)
                    ctx_tiles = {}
                    for hp in hps:
                        ctxA_ps = ps_ctx.tile([DH + 1, TQ], F32, tag=f"ctxA{hp % 2}")
                        ctxB_ps = ps_ctx.tile([DH + 1, TQ], F32, tag=f"ctxB{hp % 2}")
                        ctx_tiles[hp] = (ctxA_ps, ctxB_ps)
                    for j in range(TKC):
                        jt, jj = j // (TQ // P), j % (TQ // P)
                        for hp in hps:
                            hA, hB = 2 * hp, 2 * hp + 1
                            sc = ps_sc.tile([P, 2, TQ], F32, tag="sc")
                            nc.tensor.matmul(sc[:, 0, :],
                                             kT[jt][0:DH, hp, ts(jj, P)],
                                             qT[0:DH, hp, :],
                                             start=True, stop=True)
                            nc.tensor.matmul(sc[:, 1, :],
                                             kT[jt][DH:P, hp, ts(jj, P)],
                                             qT[DH:P, hp, :],
                                             start=True, stop=True)
                            a2 = attnp.tile([P, 2, TQ], BF16, tag="attn")
                            nc.scalar.activation(a2[:], sc[:], AF.Exp, scale=0.125,
                                                 bias=zero_col[:])
                            ctxA_ps, ctxB_ps = ctx_tiles[hp]
                            nc.tensor.matmul(ctxA_ps[:], v_sb[:, j, hA, :],
                                             a2[:, 0, :],
                                             start=(j == 0), stop=(j == TKC - 1))
                            nc.tensor.matmul(ctxB_ps[:], v_sb[:, j, hB, :],
                                             a2[:, 1, :],
                                             start=(j == 0), stop=(j == TKC - 1))
                    for hp in hps:
                        ctxA_ps, ctxB_ps = ctx_tiles[hp]
                        cuA = stgp.tile([DH + 1, TQ], F32, tag="cuA")
                        nc.scalar.copy(cuA[:], ctxA_ps[:])
                        cuB = stgp.tile([DH + 1, TQ], F32, tag="cuB")
                        nc.scalar.copy(cuB[:], ctxB_ps[:])
                        rcA = rssb.tile([DH + 1, TQ], F32, tag="rcA")
                        nc.vector.reciprocal(rcA[DH:DH + 1, :], cuA[DH:DH + 1, :])
                        rcB = rssb.tile([DH + 1, TQ], F32, tag="rcB")
                        nc.vector.reciprocal(rcB[DH:DH + 1, :], cuB[DH:DH + 1, :])
                        rbA = rbp.tile([DH, TQ], F32, tag="rbA")
                        _dma_bcast(nc, rbA, rcA[DH:DH + 1, :], DH, TQ)
                        rbB = rbp.tile([DH, TQ], F32, tag="rbB")
                        _dma_bcast(nc, rbB, rcB[DH:DH + 1, :], DH, TQ)
                        nc.vector.tensor_mul(ctxT[hp][0:DH, :], cuA[0:DH, :],
                                             rbA[:])
                        stgB = stgp.tile([DH, TQ], BF16, tag="stgB")
                        nc.vector.tensor_mul(stgB[:], cuB[0:DH, :], rbB[:])
                        nc.sync.dma_start(ctxT[hp][DH:P, :], stgB[:])

            # ---------------- Phase 3: Wo + LN2 + FFN ----------------
            with (
                tc.tile_pool(name="lat2p", bufs=1) as lat2p,
                tc.tile_pool(name="nx2p", bufs=1) as nx2p,
                tc.tile_pool(name="sq2p", bufs=1) as sq2p,
                tc.tile_pool(name="ab2p", bufs=1) as ab2p,
                tc.tile_pool(name="small2p", bufs=12) as small2p,
                tc.tile_pool(name="lntmp2p", bufs=2) as lntmp2p,
                tc.tile_pool(name="w1sp", bufs=4) as w1sp,
                tc.tile_pool(name="w2sp", bufs=4) as w2sp,
                tc.tile_pool(name="hp_pool", bufs=4) as hp_pool,
                tc.tile_pool(name="outp", bufs=1) as outp,
            ):
                lat2T = lat2p.tile([P, FC, TQ], F32)
                nx2T = []
                for cc in range(FC):
                    nx2T_c = nx2p.tile([P, TQ], BF16, tag=f"nx2T{cc}")
                    nx2T.append(nx2T_c)
                with (
                    tc.tile_pool(name="ps_wo", bufs=2, space="PSUM") as ps_wo,
                    tc.tile_pool(name="ps_st2", bufs=1, space="PSUM") as ps_st2,
                ):
                    # Wo projection + residual, LN2 stats interleaved per-chunk
                    sq2 = sq2p.tile([P, FC, TQ], BF16, tag="sq2")
                    latbf2 = sq2p.tile([P, FC, TQ], BF16, tag="latbf2")
                    ps_sum2 = ps_st2.tile([1, TQ], F32, tag="sum2")
                    ps_sq2 = ps_st2.tile([33, TQ], F32, tag="sqs2")
                    for mc in range(FC):
                        ps = ps_wo.tile([P, TQ], F32, tag="wo")
                        for kc in range(FC):
                            nc.tensor.matmul(ps[:], wo_sb[:, kc, ts(mc, P)],
                                             ctxT[kc][:],
                                             start=(kc == 0), stop=(kc == FC - 1))
                        nc.vector.affine_then_add(lat2T[:, mc, :], ps[:],
                                                  resid1[:, mc, :], 1.0,
                                                  bo_sb[:, mc:mc + 1])
                        nc.scalar.copy(latbf2[:, mc, :], lat2T[:, mc, :])
                        nc.vector.tensor_mul(sq2[:, mc, :], lat2T[:, mc, :],
                                             lat2T[:, mc, :])
                        nc.tensor.matmul(ps_sum2[0:1, :], ones_col_bf[:],
                                         latbf2[:, mc, :],
                                         start=(mc == 0), stop=(mc == FC - 1))
                        nc.tensor.matmul(ps_sq2[32:33, :], ones_col_bf[:],
                                         sq2[:, mc, :],
                                         start=(mc == 0), stop=(mc == FC - 1))
                    ab2, bb2 = _ln_tail(nc, TQ, ps_sum2[0:1, :],
                                        ps_sq2[32:33, :], small2p, ab2p,
                                        eps_tile)
                    for c in range(FC):
                        t2 = lntmp2p.tile([P, TQ], F32, tag="lntmp2")
                        nc.vector.tensor_mul(t2[:], lat2T[:, c, :], ab2[:])
                        nc.vector.tensor_add(nx2T[c][:], t2[:], bb2[:])

                outT = outp.tile([P, FC, TQ], F32)
                with (
                    tc.tile_pool(name="ps_fo", bufs=1, space="PSUM") as ps_fo,
                    tc.tile_pool(name="ps_h", bufs=2, space="PSUM") as ps_h,
                ):
                    ps_out = ps_fo.tile([P, FC, TQ], F32)
                    for mh in range(FFC):
                        w1t = w1sp.tile([P, FC, P], BF16, tag="w1s")
                        nc.sync.dma_start(
                            w1t[:], w1_d.ap()[mh].rearrange("(c p) m -> p c m", p=P))
                        w2t = w2sp.tile([P, H], BF16, tag="w2s")
                        nc.sync.dma_start(w2t[:], w2_d.ap()[ts(mh, P)])
                        psh = ps_h.tile([P, TQ], F32, tag="h")
                        for kc in range(FC):
                            nc.tensor.matmul(psh[:], w1t[:, kc, :], nx2T[kc][:],
                                             start=(kc == 0), stop=(kc == FC - 1))
                        h_t = hp_pool.tile([P, TQ], BF16, tag="h_sb")
                        nc.scalar.activation(h_t[:], psh[:], AF.Gelu,
                                             bias=b1_sb[:, mh:mh + 1])
                        for mc in range(FC):
                            nc.tensor.matmul(ps_out[:, mc, :], w2t[:, ts(mc, P)],
                                             h_t[:],
                                             start=(mh == 0), stop=(mh == FFC - 1))
                    for mc in range(FC):
                        nc.vector.affine_then_add(outT[:, mc, :], ps_out[:, mc, :],
                                                  lat2T[:, mc, :], 1.0,
                                                  b2_sb[:, mc:mc + 1])
                nc.sync.dma_start(out_ap, outT[:])

    nc.compile()
    return nc


_NC_CACHE = {}


def _get_nc():
    if "nc" not in _NC_CACHE:
        _NC_CACHE["nc"] = build()
    return _NC_CACHE["nc"]


def _prep_inputs(latent, ln1_w, ln1_b, Wq, bq, Wk, bk, Wv, bv, Wo, bo,
                 ln2_w, ln2_b, W1, b1, W2, b2):
    f32 = np.float32
    bf16 = ml_dtypes.bfloat16
    lat = np.asarray(latent, f32)
    ln1_w = np.asarray(ln1_w, f32); ln1_b = np.asarray(ln1_b, f32)
    ln2_w = np.asarray(ln2_w, f32); ln2_b = np.asarray(ln2_b, f32)
    Wq = np.asarray(Wq, f32); Wk = np.asarray(Wk, f32); Wv = np.asarray(Wv, f32)
    Wo = np.asarray(Wo, f32); W1 = np.asarray(W1, f32); W2 = np.asarray(W2, f32)
    bq = np.asarray(bq, f32); bk = np.asarray(bk, f32); bv = np.asarray(bv, f32)
    bo = np.asarray(bo, f32); b1 = np.asarray(b1, f32); b2 = np.asarray(b2, f32)

    wq_eff = (ln1_w[:, None] * Wq).astype(bf16)
    wk_eff = (ln1_w[:, None] * Wk).astype(bf16)
    wv_eff = (ln1_w[:, None] * Wv).astype(bf16)
    wo_bf = Wo.astype(bf16)
    bq_eff = ln1_b @ Wq + bq
    bk_eff = ln1_b @ Wk + bk
    bv_eff = ln1_b @ Wv + bv
    bo_eff = bv_eff @ Wo + bo
    w1_eff = ln2_w[:, None] * W1
    b1_eff = ln2_b @ W1 + b1
    w1r = np.ascontiguousarray(
        w1_eff.reshape(H, FFC, P).transpose(1, 0, 2)).astype(bf16)
    w2_bf = W2.astype(bf16)

    def chunked(b):  # [H or FF] -> [P, nchunks]
        return np.ascontiguousarray(b.reshape(-1, P).T)

    common = {
        "wq": wq_eff, "wk": wk_eff, "wv": wv_eff, "wo": wo_bf,
        "w1r": w1r, "w2": w2_bf,
        "bq": chunked(bq_eff), "bk": chunked(bk_eff), "bo": chunked(bo_eff),
        "b1": chunked(b1_eff), "b2": chunked(b2),
    }
    in_maps = []
    for c in range(NCORES):
        b = c // (NCORES // B)
        q = c % (NCORES // B)
        latT_c = np.ascontiguousarray(np.roll(lat[b].T, -q * TQ, axis=1))
        m = dict(common)
        m["latTq"] = np.ascontiguousarray(latT_c[:, :TQ])
        m["latTbf"] = latT_c.astype(bf16)
        in_maps.append(m)
    return in_maps


def kernel(**inputs):
    nc = _get_nc()
    in_maps = _prep_inputs(**inputs)
    res = run_bass_kernel_spmd(nc, in_maps, core_ids=list(range(NCORES)))
    out = np.empty((B, S, H), np.float32)
    for c in range(NCORES):
        b = c // (NCORES // B)
        q = c % (NCORES // B)
        out[b, q * TQ:(q + 1) * TQ, :] = res.results[c]["outT"].T
    return out
